# revision 9
# baseline (speedup 1.0000x reference)
"""Trainium2 Bass kernel for a NAM (per-feature MLP ensemble).

Model: 64 independent per-feature MLPs (1 -> 64 -> 64 -> 64 -> 1), summed with
w_final and passed through sigmoid.  B=16384, F=64, H=64, L=2 hidden layers.

Strategy (8 cores, data-parallel over batch, 2048 rows/core):
  - Activations live as feature-pair tiles [128 partitions = 2 features x 64
    hidden dims, batch on the free dim].  All matmuls are weight-stationary
    with the batch streaming (N=512 per pass), dtype float32r (full-rate fp32
    on the PE with reduced-precision multiplies).
  - Input layer is computed on the PE: rhs partitions hold interleaved
    (x_j, 1) rows so w_in*x + b_in comes out of one matmul per pair.
  - Hidden layers: per pair, one K=128 M=128 matmul with block-diagonal
    weights (two independent 64x64 feature nets per pass).
  - Output layer is algebraically folded: v = w_final[:,None]*w_out; |v| is
    folded into layer-2 weights/biases, sign(v) goes into an accumulating
    K=128 M=2 reduction matmul into one PSUM bank; the constant term
    sum(w_final*b_out)+b_final becomes the sigmoid bias on the ScalarE.
  - Each relu is one fused PSUM->SBUF op (tensor_scalar add+max on DVE or
    activation(Relu, bias) on ACT), engines assigned by role for balance.
"""

import numpy as np

F = 64
H = 64
B = 16384
N_CORES = 8
BL = B // N_CORES          # 2048 rows per core
N_MEGA = 2                 # megas per core
N_HALF = 2                 # 512-column halves per mega
COLS = 512                 # matmul streaming width
N_PAIR = F // 2            # 32 feature pairs

# packed f32r const layout (columns in wpkr)
_WIN0 = 0                  # win_full   [128, 32, 128]
_WH1 = _WIN0 + 32 * 128    # wh1_full   [128, 32, 128]
_WH2 = _WH1 + 32 * 128     # wh2_full   [128, 32, 128]
_SGN = _WH2 + 32 * 128     # sgn        [128, 32, 3]
_WPKR_COLS = _SGN + 32 * 3
# packed f32 const layout (columns in wpk)
_B1 = 0
_B2 = 32
_C0 = 64
_WPK_COLS = 65

_cache = {}


def _build_program():
    import concourse.bass as bass
    import concourse.mybir as mybir
    import concourse.tile as tile
    from concourse import bacc
    from contextlib import ExitStack

    f32 = mybir.dt.float32
    f32r = mybir.dt.float32r
    Alu = mybir.AluOpType
    Act = mybir.ActivationFunctionType

    nc = bacc.Bacc("TRN2", target_bir_lowering=False, debug=False)

    xin_d = nc.declare_dram_parameter("xin", [128, N_MEGA, N_HALF, COLS], f32r, isOutput=False)
    wpkr_d = nc.declare_dram_parameter("wpkr", [128, _WPKR_COLS], f32r, isOutput=False)
    wpk_d = nc.declare_dram_parameter("wpk", [128, _WPK_COLS], f32, isOutput=False)
    out_d = nc.declare_dram_parameter("out", [N_MEGA, N_HALF, COLS], f32, isOutput=True)

    with ExitStack() as ctx:
        tc = ctx.enter_context(tile.TileContext(nc))
        consts = ctx.enter_context(tc.tile_pool(name="consts", bufs=1))
        h0_pool = ctx.enter_context(tc.tile_pool(name="h0", bufs=2))
        h1_pool = ctx.enter_context(tc.tile_pool(name="h1", bufs=2))
        h2_pool = ctx.enter_context(tc.tile_pool(name="h2", bufs=2))
        outp = ctx.enter_context(tc.tile_pool(name="outp", bufs=2))
        pre0_pool = ctx.enter_context(tc.tile_pool(name="pre0", bufs=1, space="PSUM"))
        pre1_pool = ctx.enter_context(tc.tile_pool(name="pre1", bufs=1, space="PSUM"))
        pre2_pool = ctx.enter_context(tc.tile_pool(name="pre2", bufs=1, space="PSUM"))
        red_pool = ctx.enter_context(tc.tile_pool(name="red", bufs=2, space="PSUM"))

        wpkr_sb = consts.tile([128, _WPKR_COLS], f32r)
        nc.sync.dma_start(out=wpkr_sb, in_=wpkr_d[:])
        xin_sb = consts.tile([128, N_MEGA, N_HALF, COLS], f32r)
        nc.sync.dma_start(out=xin_sb, in_=xin_d[:])
        wpk_sb = consts.tile([128, _WPK_COLS], f32)
        nc.sync.dma_start(out=wpk_sb, in_=wpk_d[:])

        win_v = wpkr_sb[:, _WIN0:_WH1].rearrange("p (q m) -> p q m", q=N_PAIR)
        wh1_v = wpkr_sb[:, _WH1:_WH2].rearrange("p (q m) -> p q m", q=N_PAIR)
        wh2_v = wpkr_sb[:, _WH2:_SGN].rearrange("p (q m) -> p q m", q=N_PAIR)
        sgn_v = wpkr_sb[:, _SGN:_WPKR_COLS].rearrange("p (q t) -> p q t", q=N_PAIR)
        b1_v = wpk_sb[:, _B1:_B2]
        b2_v = wpk_sb[:, _B2:_C0]
        c0_v = wpk_sb[0:2, _C0:_C0 + 1]

        def evac_dve(out_ap, in_ap, bias_ap):
            if bias_ap is None:
                nc.vector.tensor_scalar(out_ap, in_ap, 0.0, None, Alu.max)
            else:
                nc.vector.tensor_scalar(out_ap, in_ap, bias_ap, 0.0, Alu.add, Alu.max)

        def evac_act(out_ap, in_ap, bias_ap):
            nc.scalar.activation(out_ap, in_ap, Act.Relu,
                                 bias=0.0 if bias_ap is None else bias_ap,
                                 scale=1.0)

        for m in range(N_MEGA):
            red = red_pool.tile([2, COLS], f32)

            # software pipeline over pairs: stage offsets keep PE busy while
            # DVE/ACT evacuate earlier pairs' PSUM banks
            st_in, st_h1, st_h2 = {}, {}, {}
            for t in range(N_PAIR + 3):
                q = t
                if q < N_PAIR:
                    pre0 = pre0_pool.tile([128, N_HALF, COLS], f32, tag="pre0")
                    for hf in range(N_HALF):
                        nc.tensor.matmul(
                            pre0[:, hf, :],
                            win_v[:, q, :],
                            xin_sb[:, m, hf, :],
                            start=True, stop=True,
                        )
                    st_in[q] = pre0

                q = t - 1
                if 0 <= q < N_PAIR:
                    pre0 = st_in.pop(q)
                    h0 = h0_pool.tile([128, N_HALF, COLS], f32r, tag="h0")
                    # split evac0 between engines for load balance
                    (evac_dve if q % 5 < 2 else evac_act)(h0[:], pre0[:], None)
                    pre1 = pre1_pool.tile([128, N_HALF, COLS], f32, tag="pre1")
                    for hf in range(N_HALF):
                        nc.tensor.matmul(
                            pre1[:, hf, :],
                            wh1_v[:, q, :],
                            h0[:, hf, :],
                            start=True, stop=True,
                        )
                    st_h1[q] = pre1

                q = t - 2
                if 0 <= q < N_PAIR:
                    pre1 = st_h1.pop(q)
                    h1 = h1_pool.tile([128, N_HALF, COLS], f32r, tag="h1")
                    evac_dve(h1[:], pre1[:], b1_v[:, q:q + 1])
                    pre2 = pre2_pool.tile([128, N_HALF, COLS], f32, tag="pre2")
                    for hf in range(N_HALF):
                        nc.tensor.matmul(
                            pre2[:, hf, :],
                            wh2_v[:, q, :],
                            h1[:, hf, :],
                            start=True, stop=True,
                        )
                    st_h2[q] = pre2

                q = t - 3
                if 0 <= q < N_PAIR:
                    pre2 = st_h2.pop(q)
                    h2 = h2_pool.tile([128, N_HALF, COLS], f32r, tag="h2")
                    evac_act(h2[:], pre2[:], b2_v[:, q:q + 1])
                    for hf in range(N_HALF):
                        nc.tensor.matmul(
                            red[:],
                            sgn_v[:, q, hf:hf + 2],
                            h2[:, hf, :],
                            start=(q == 0 and hf == 0),
                            stop=(q == N_PAIR - 1 and hf == 1),
                        )

            out_sb = outp.tile([2, COLS], f32, tag="out")
            nc.scalar.activation(out_sb[:], red[:], Act.Sigmoid,
                                 bias=c0_v, scale=1.0)
            nc.sync.dma_start(out=out_d[m], in_=out_sb)

    nc.compile()
    return nc


def _prep_weights(w_in, b_in, w_hid, b_hid, w_out, b_out, w_final, b_final):
    f32 = np.float32
    v = (w_final[:, None] * w_out).astype(f32)            # [F, H]
    av = np.abs(v)
    sg = np.where(v >= 0, 1.0, -1.0).astype(f32)
    c0 = float(np.dot(w_final, b_out) + b_final[0])

    win = np.zeros((128, N_PAIR, 128), f32)
    wh1 = np.zeros((128, N_PAIR, 128), f32)
    wh2 = np.zeros((128, N_PAIR, 128), f32)
    sgn = np.zeros((128, N_PAIR, 3), f32)
    wpk = np.zeros((128, _WPK_COLS), f32)
    for q in range(N_PAIR):
        a, b = 2 * q, 2 * q + 1
        # input layer: rhs rows are interleaved (x_j, 1); only features
        # a, b contribute to this pair's 128 output columns
        win[2 * a + 0, q, 0:64] = w_in[a]
        win[2 * a + 1, q, 0:64] = b_in[a]
        win[2 * b + 0, q, 64:128] = w_in[b]
        win[2 * b + 1, q, 64:128] = b_in[b]
        # hidden layers: block-diagonal pair weights
        wh1[0:64, q, 0:64] = w_hid[0, a]
        wh1[64:128, q, 64:128] = w_hid[0, b]
        wh2[0:64, q, 0:64] = w_hid[1, a] * av[a][None, :]
        wh2[64:128, q, 64:128] = w_hid[1, b] * av[b][None, :]
        # reduction signs: columns [s, 0, s] so slice [h:h+2] gives
        # [s, 0] for half 0 and [0, s] for half 1
        sgn[0:64, q, 0] = sg[a]
        sgn[64:128, q, 0] = sg[b]
        sgn[0:64, q, 2] = sg[a]
        sgn[64:128, q, 2] = sg[b]
        wpk[0:64, _B1 + q] = b_hid[0, a]
        wpk[64:128, _B1 + q] = b_hid[0, b]
        wpk[0:64, _B2 + q] = b_hid[1, a] * av[a]
        wpk[64:128, _B2 + q] = b_hid[1, b] * av[b]
    wpk[:, _C0] = c0

    wpkr = np.concatenate(
        [win.reshape(128, -1), wh1.reshape(128, -1), wh2.reshape(128, -1),
         sgn.reshape(128, -1)], axis=1)
    assert wpkr.shape[1] == _WPKR_COLS
    return dict(wpkr=np.ascontiguousarray(wpkr), wpk=wpk)


def _make_in_maps(x, weights):
    in_maps = []
    for c in range(N_CORES):
        xc = np.ascontiguousarray(x[c * BL:(c + 1) * BL])        # [BL, F]
        xT = np.ascontiguousarray(xc.T).reshape(F, N_MEGA, N_HALF, COLS)
        xin = np.empty((128, N_MEGA, N_HALF, COLS), np.float32)
        xin[0::2] = xT
        xin[1::2] = 1.0
        m = dict(weights)
        m["xin"] = xin
        in_maps.append(m)
    return in_maps


def _fix_pe_waits(nc):
    """walrus's S3_LW (ldweights/matmul) sync-wait field holds at most 2
    waits; the Tile scheduler occasionally emits 3.  Move the excess onto a
    no-fuse NoOp inserted right before the instruction (the waits still
    happen-before the matmul on the same sequencer)."""
    import concourse.mybir as mybir

    nfixed = 0
    for blk in nc.m.functions[0].blocks:
        out = []
        changed = False
        for i in blk.instructions:
            tn = type(i).__name__
            si = i.sync_info
            if ("Matmult" in tn or "Ldweights" in tn) and si and si.on_wait \
                    and len(si.on_wait) > 2:
                waits = list(si.on_wait)
                nop = mybir.InstNoOp(
                    name=f"{i.name}-wn", engine=i.engine, bass_nofuse=True,
                    ins=[], outs=[],
                    sync_info=mybir.SyncInfo(on_wait=waits[:-2], on_update=[]),
                )
                i.sync_info = mybir.SyncInfo(
                    on_wait=waits[-2:], on_update=list(si.on_update))
                out.append(nop)
                nfixed += 1
                changed = True
            out.append(i)
        if changed:
            blk.instructions = out
    return nfixed


def _get_program():
    if "nc" not in _cache:
        _cache["nc"] = _build_program()
    return _cache["nc"]


def run(inputs, trace=False, **trace_kwargs):
    from concourse.bass_utils import run_bass_kernel_spmd

    x = np.asarray(inputs["x"], np.float32)
    weights = _prep_weights(
        np.asarray(inputs["w_in"], np.float32),
        np.asarray(inputs["b_in"], np.float32),
        np.asarray(inputs["w_hid"], np.float32),
        np.asarray(inputs["b_hid"], np.float32),
        np.asarray(inputs["w_out"], np.float32),
        np.asarray(inputs["b_out"], np.float32),
        np.asarray(inputs["w_final"], np.float32),
        np.asarray(inputs["b_final"], np.float32),
    )
    nc = _get_program()
    in_maps = _make_in_maps(x, weights)
    res = run_bass_kernel_spmd(nc, in_maps, list(range(N_CORES)),
                               trace=trace, **trace_kwargs)
    outs = [res.results[c]["out"].reshape(BL) for c in range(N_CORES)]
    out = np.concatenate(outs).reshape(B, 1).astype(np.float32)
    return out, res


def kernel(**inputs):
    out, _ = run(inputs, trace=False)
    return out
